# revision 23
# baseline (speedup 1.0000x reference)
"""Causal multi-head attention layer on 8 trn2 NeuronCores.

Sharding: 8 cores = 4 batches x 2 head-groups. Core c handles batch c//2 and
heads [8*(c%2), 8*(c%2)+8). Each core runs QKV projections for its 512-wide
head slice, causal flash attention for 8 heads, and a partial output
projection (its 512 rows of Wo). Host sums the two partials per batch + bo.

Problem constants (hardcoded per contract): B=4, L=2048, D=1024, H=16, DK=DV=64.
"""

import sys

for _p in ("/opt/trn_rl_repo",):
    if _p not in sys.path:
        sys.path.insert(0, _p)

import numpy as np
import ml_dtypes

import concourse.bass as bass
import concourse.tile as tile
from concourse import bacc, mybir
from concourse.bass_utils import run_bass_kernel_spmd
BF16 = ml_dtypes.bfloat16

B, L, D, H, DK, DV = 4, 2048, 1024, 16, 64, 64
N_CORES = 8
HL = 8          # heads per core
DH = 512        # local head dim (HL * DK)
P = 128
LC = 512        # l-chunk
NLC = L // LC   # 4
NDC = D // P    # 8 contraction chunks for projections
NKC = DH // P   # 4 dk chunks
NST = L // P    # 16 s tiles
VW = DV + 1     # 65: V columns + ones column
SCALE = 1.0 / np.sqrt(DK)
MASK_NEG = -1.0e5

TRACE = False          # set by test harness for profiling runs
LAST_RESULTS = None    # BassKernelResults of the last run (for profiling)

STAGE = 3              # debug: 1=projections only, 2=+attention, 3=full

_COMPILED = None


def _build():
    f32 = mybir.dt.float32
    bf16 = mybir.dt.bfloat16
    AF = mybir.ActivationFunctionType

    nc = bacc.Bacc("TRN2", target_bir_lowering=False, debug=False,
                   num_devices=N_CORES)

    xqT = nc.dram_tensor("xqT", [NLC, P, NDC, LC], bf16, kind="ExternalInput").ap()
    xkT = nc.dram_tensor("xkT", [NLC, P, NDC, LC], bf16, kind="ExternalInput").ap()
    xvT = nc.dram_tensor("xvT", [NLC, P, NDC, LC], bf16, kind="ExternalInput").ap()
    wq = nc.dram_tensor("wq", [P, NDC, DH], bf16, kind="ExternalInput").ap()
    wk = nc.dram_tensor("wk", [P, NDC, DH], bf16, kind="ExternalInput").ap()
    wv = nc.dram_tensor("wv", [P, NDC, DH], bf16, kind="ExternalInput").ap()
    wo = nc.dram_tensor("wo", [P, NKC, D], bf16, kind="ExternalInput").ap()
    bq = nc.dram_tensor("bq", [P, NKC], f32, kind="ExternalInput").ap()
    bk = nc.dram_tensor("bk", [P, NKC], f32, kind="ExternalInput").ap()
    bv = nc.dram_tensor("bv", [P, DH], f32, kind="ExternalInput").ap()
    outp = nc.dram_tensor("outp", [L, D], f32, kind="ExternalOutput").ap()

    from contextlib import ExitStack

    with tile.TileContext(nc) as tc, ExitStack() as ctx:
        const = ctx.enter_context(tc.tile_pool(name="const", bufs=1))
        kvp = ctx.enter_context(tc.tile_pool(name="kv", bufs=1))
        xp = ctx.enter_context(tc.tile_pool(name="x", bufs=2))
        qp = ctx.enter_context(tc.tile_pool(name="qt", bufs=2))
        ptp = ctx.enter_context(tc.tile_pool(name="pt", bufs=4))
        atp = ctx.enter_context(tc.tile_pool(name="at", bufs=2))
        osb = ctx.enter_context(tc.tile_pool(name="osb", bufs=2))
        nrm = ctx.enter_context(tc.tile_pool(name="nrm", bufs=3))
        ps_pj = ctx.enter_context(tc.tile_pool(name="ps_pj", bufs=2, space="PSUM"))
        ps_s = ctx.enter_context(tc.tile_pool(name="ps_s", bufs=4, space="PSUM"))
        ps_o = ctx.enter_context(tc.tile_pool(name="ps_o", bufs=2, space="PSUM"))

        # ---- constants (wq + first x chunk issued first; see pipeline tail) ----
        wq_sb = const.tile([P, NDC, DH], bf16, tag="wq")
        nc.sync.dma_start(wq_sb[:], wq[:])
        x0 = []
        for nm, dram in (("xq", xqT), ("xk", xkT), ("xv", xvT)):
            t = xp.tile([P, NDC, LC], bf16, tag=nm)
            nc.sync.dma_start(t[:], dram[0])
            x0.append(t)
        bq_sb = const.tile([P, NKC], f32, tag="bq")
        nc.sync.dma_start(bq_sb[:], bq[:])
        wk_sb = const.tile([P, NDC, DH], bf16, tag="wk")
        nc.sync.dma_start(wk_sb[:], wk[:])
        bk_sb = const.tile([P, NKC], f32, tag="bk")
        nc.sync.dma_start(bk_sb[:], bk[:])
        wv_sb = const.tile([P, NDC, DH], bf16, tag="wv")
        nc.sync.dma_start(wv_sb[:], wv[:])
        bv_sb = const.tile([P, DH], f32, tag="bv")
        nc.sync.dma_start(bv_sb[:], bv[:])
        wo_sb = const.tile([P, NKC, D], bf16, tag="wo")
        nc.sync.dma_start(wo_sb[:], wo[:])

        # additive causal mask for diagonal 128x128 blocks of S^T (s part, l free):
        # keep (0) where s <= l, MASK_NEG where s > l
        cmask = const.tile([P, P], f32, tag="cmask")
        nc.gpsimd.memset(cmask[:], 0.0)
        nc.gpsimd.affine_select(
            out=cmask[:], in_=cmask[:],
            compare_op=mybir.AluOpType.is_ge,
            fill=MASK_NEG, base=0,
            pattern=[[1, P]], channel_multiplier=-1,
        )

        # persistent K^T (dk, s) and V (s, dv+ones) for the whole core
        kT_sb = kvp.tile([P, NKC, L], bf16, tag="kT")
        v_sb = kvp.tile([P, NST, HL * VW], bf16, tag="v")
        ones_view = v_sb[:].rearrange("p t (h c) -> p t h c", c=VW)[:, :, :, DV:]
        nc.vector.memset(ones_view, 1.0)  # the appended ones columns

        def load_and_project(lc, preloaded=None):
            lsl = bass.ts(lc, LC)  # this chunk's l columns

            # ---- load transposed input chunks ----
            if preloaded is not None:
                xq_t, xk_t, xv_t = preloaded
            else:
                xq_t = xp.tile([P, NDC, LC], bf16, tag="xq")
                nc.sync.dma_start(xq_t[:], xqT[lc])
                xk_t = xp.tile([P, NDC, LC], bf16, tag="xk")
                nc.sync.dma_start(xk_t[:], xkT[lc])
                xv_t = xp.tile([P, NDC, LC], bf16, tag="xv")
                nc.sync.dma_start(xv_t[:], xvT[lc])

            # ---- q/k projections: qT[dk, l] = (x @ W)^T ----
            qt_t = qp.tile([P, NKC, LC], bf16, tag="qt")
            for kc in range(NKC):
                ps = ps_pj.tile([P, LC], f32, tag="ps_pj")
                for dc in range(NDC):
                    nc.tensor.matmul(ps[:], wq_sb[:, dc, bass.ts(kc, P)],
                                     xq_t[:, dc, :],
                                     start=(dc == 0), stop=(dc == NDC - 1))
                nc.vector.tensor_scalar_add(qt_t[:, kc, :], ps[:],
                                            bq_sb[:, kc:kc + 1])
            for kc in range(NKC):
                ps = ps_pj.tile([P, LC], f32, tag="ps_pj")
                for dc in range(NDC):
                    nc.tensor.matmul(ps[:], wk_sb[:, dc, bass.ts(kc, P)],
                                     xk_t[:, dc, :],
                                     start=(dc == 0), stop=(dc == NDC - 1))
                nc.vector.tensor_scalar_add(kT_sb[:, kc, lsl], ps[:],
                                            bk_sb[:, kc:kc + 1])

            # ---- v projection: V[s, dv] = x @ Wv + bv ----
            for j in range(LC // P):
                st = lc * (LC // P) + j
                ps = ps_pj.tile([P, LC], f32, tag="ps_pj")
                for dc in range(NDC):
                    nc.tensor.matmul(ps[:], xv_t[:, dc, bass.ts(j, P)],
                                     wv_sb[:, dc, :],
                                     start=(dc == 0), stop=(dc == NDC - 1))
                vv = v_sb[:, st, :].rearrange("p (h c) -> p h c", c=VW)[:, :, :DV]
                nc.vector.tensor_tensor(
                    vv, ps[:].rearrange("p (h c) -> p h c", c=DV),
                    bv_sb[:].rearrange("p (h c) -> p h c", c=DV),
                    mybir.AluOpType.add)
            return qt_t

        def attention(lc, qt_t):
            n_st = (lc + 1) * (LC // P)
            at_t = [atp.tile([P, LC], bf16, tag=f"at{hc}", name=f"at{hc}") for hc in range(NKC)]
            for hp in range(NKC):
                h0, h1 = 2 * hp, 2 * hp + 1
                po0 = ps_o.tile([P, LC], f32, tag="ps_o")
                po1 = ps_o.tile([P, LC], f32, tag="ps_o")

                def mm1(st):
                    # S^T = K(dk,s).T' @ qT : two heads packed on PE rows
                    jj = st - lc * (LC // P)
                    nc0 = jj * P if jj >= 0 else 0  # first valid l col
                    s0 = ps_s.tile([P, LC], f32, tag="ps_s")
                    s1 = ps_s.tile([P, LC], f32, tag="ps_s")
                    nc.tensor.matmul(s0[:, nc0:], kT_sb[0:64, hp, bass.ts(st, P)],
                                     qt_t[0:64, hp, nc0:], start=True, stop=True,
                                     tile_position=(0, 0))
                    nc.tensor.matmul(s1[:, nc0:], kT_sb[64:128, hp, bass.ts(st, P)],
                                     qt_t[64:128, hp, nc0:], start=True, stop=True,
                                     tile_position=(64, 0))
                    if jj >= 0:  # diagonal block: additive causal mask
                        nc.vector.tensor_tensor(s0[:, nc0:nc0 + P], s0[:, nc0:nc0 + P],
                                                cmask[:], mybir.AluOpType.add)
                        nc.vector.tensor_tensor(s1[:, nc0:nc0 + P], s1[:, nc0:nc0 + P],
                                                cmask[:], mybir.AluOpType.add)
                    return s0, s1, nc0

                pend = mm1(0)
                for st in range(n_st):
                    s0, s1, nc0 = pend
                    if st + 1 < n_st:
                        pend = mm1(st + 1)  # PE runs one step ahead of ACT
                    # P^T = exp(scale * S^T), bf16
                    pt0 = ptp.tile([P, LC], bf16, tag="pt")
                    pt1 = ptp.tile([P, LC], bf16, tag="pt")
                    nc.scalar.activation(pt0[:, nc0:], s0[:, nc0:], AF.Exp,
                                         bias=0.0, scale=float(SCALE))
                    nc.scalar.activation(pt1[:, nc0:], s1[:, nc0:], AF.Exp,
                                         bias=0.0, scale=float(SCALE))
                    # out^T[(dv|sum), l] += V_aug.T @ P^T
                    nc.tensor.matmul(po0[0:VW, nc0:], v_sb[:, st, h0 * VW:(h0 + 1) * VW],
                                     pt0[:, nc0:],
                                     start=(st == 0), stop=(st == n_st - 1))
                    nc.tensor.matmul(po1[0:VW, nc0:], v_sb[:, st, h1 * VW:(h1 + 1) * VW],
                                     pt1[:, nc0:],
                                     start=(st == 0), stop=(st == n_st - 1))

                # evict psum immediately: rows 0:64 unnorm out^T, row 64 denom
                un0 = nrm.tile([P, LC], f32, tag="un")
                nc.vector.tensor_copy(un0[0:VW, :], po0[0:VW, :])
                un1 = nrm.tile([P, LC], f32, tag="un")
                nc.vector.tensor_copy(un1[0:VW, :], po1[0:VW, :])

                # normalize off-psum
                rz0 = nrm.tile([1, LC], f32, tag="rz")
                nc.sync.dma_start(rz0[:], un0[64:65, :])
                rr0 = nrm.tile([1, LC], f32, tag="rr")
                nc.vector.reciprocal_approx_fast(rr0[:], rz0[:])
                rb0 = nrm.tile([64, LC], f32, tag="rb")
                nc.gpsimd.partition_broadcast(rb0[:], rr0[:])
                nc.vector.tensor_mul(at_t[hp][0:64, :], un0[0:64, :], rb0[:])

                rz1 = nrm.tile([1, LC], f32, tag="rz")
                nc.sync.dma_start(rz1[:], un1[64:65, :])
                rr1 = nrm.tile([1, LC], f32, tag="rr")
                nc.vector.reciprocal_approx_fast(rr1[:], rz1[:])
                rb1 = nrm.tile([64, LC], f32, tag="rb")
                nc.gpsimd.partition_broadcast(rb1[:], rr1[:])
                tmp1 = nrm.tile([64, LC], bf16, tag="tmp1")
                nc.vector.tensor_mul(tmp1[:], un1[0:64, :], rb1[:])
                nc.sync.dma_start(at_t[hp][64:128, :], tmp1[:])
            return at_t

        def out_proj(lc, at_t):
            for lt in range(LC // P):
                o_sb = osb.tile([P, D], f32, tag="o_sb")
                for n in range(2):
                    ps = ps_pj.tile([P, LC], f32, tag="ps_pj")
                    for hc in range(NKC):
                        nc.tensor.matmul(ps[:], at_t[hc][:, bass.ts(lt, P)],
                                         wo_sb[:, hc, bass.ts(n, 512)],
                                         start=(hc == 0), stop=(hc == NKC - 1))
                    nc.vector.tensor_copy(o_sb[:, bass.ts(n, 512)], ps[:])
                nc.sync.dma_start(outp[lc * LC + lt * P: lc * LC + (lt + 1) * P, :],
                                  o_sb[:])

        # software pipeline: projections of lc+1 overlap the normalize tail
        # of attention(lc), keeping PE busy across the lc boundary
        qt_cur = load_and_project(0, preloaded=x0)
        at_prev = None
        for lc in range(NLC):
            at_t = attention(lc, qt_cur)
            if at_prev is not None:
                out_proj(lc - 1, at_prev)
            if lc + 1 < NLC:
                qt_cur = load_and_project(lc + 1)
            at_prev = at_t
        out_proj(NLC - 1, at_prev)

    nc.compile()
    return nc


def _get_compiled():
    global _COMPILED
    if _COMPILED is None:
        _COMPILED = _build()
    return _COMPILED


def kernel(queries, keys, values, Wq, bq, Wk, bk, Wv, bv, Wo, bo):
    global LAST_RESULTS
    nc = _get_compiled()

    queries = np.asarray(queries, np.float32)
    keys = np.asarray(keys, np.float32)
    values = np.asarray(values, np.float32)

    def pack_x(x):
        # (L, D) -> (NLC, P, NDC, LC): [lc, p, dc, l] = x[lc*LC+l, dc*P+p]
        t = x.T.reshape(NDC, P, NLC, LC)          # [dc, p, lc, l]
        return np.ascontiguousarray(t.transpose(2, 1, 0, 3)).astype(BF16)

    xT = {}
    for b in range(B):
        xT[("q", b)] = pack_x(np.asarray(queries[b]))
        xT[("k", b)] = pack_x(np.asarray(keys[b]))
        xT[("v", b)] = pack_x(np.asarray(values[b]))

    wslice = {}
    for g in range(2):
        sl = slice(DH * g, DH * (g + 1))
        def pack_w(w):
            # (D, DH) -> (P, NDC, DH)
            return np.ascontiguousarray(
                w.reshape(NDC, P, DH).transpose(1, 0, 2)).astype(BF16)

        wslice[("wq", g)] = pack_w(np.asarray(Wq, np.float32)[:, sl])
        wslice[("wk", g)] = pack_w(np.asarray(Wk, np.float32)[:, sl])
        wslice[("wv", g)] = pack_w(np.asarray(Wv, np.float32)[:, sl])
        wslice[("wo", g)] = np.ascontiguousarray(
            np.asarray(Wo, np.float32)[sl, :].reshape(NKC, P, D).transpose(1, 0, 2)
        ).astype(BF16)
        wslice[("bq", g)] = np.ascontiguousarray(
            np.asarray(bq, np.float32)[sl].reshape(NKC, P).T)
        wslice[("bk", g)] = np.ascontiguousarray(
            np.asarray(bk, np.float32)[sl].reshape(NKC, P).T)
        wslice[("bv", g)] = np.ascontiguousarray(
            np.broadcast_to(np.asarray(bv, np.float32)[sl], (P, DH)))

    in_maps = []
    for c in range(N_CORES):
        b, g = c // 2, c % 2
        in_maps.append({
            "xqT": xT[("q", b)], "xkT": xT[("k", b)], "xvT": xT[("v", b)],
            "wq": wslice[("wq", g)], "wk": wslice[("wk", g)],
            "wv": wslice[("wv", g)], "wo": wslice[("wo", g)],
            "bq": wslice[("bq", g)], "bk": wslice[("bk", g)],
            "bv": wslice[("bv", g)],
        })

    res = run_bass_kernel_spmd(nc, in_maps, list(range(N_CORES)), trace=TRACE)
    LAST_RESULTS = res

    bo32 = np.asarray(bo, np.float32)
    out = np.empty((B, L, D), np.float32)
    for b in range(B):
        out[b] = res.results[2 * b]["outp"] + res.results[2 * b + 1]["outp"] + bo32
    return out
